# revision 56
# baseline (speedup 1.0000x reference)
"""Trainium2 Bass kernel for LinearSelfAttention3D (16x256x64x64, 8 heads, mem_kv).

Data-parallel over batch: 2 batches per core, 8 cores, identical SPMD program.
Per batch (x viewed [256, 4096] channel-major):
  Pass A (32 s-tiles of 128, ctx matmuls software-pipelined one tile behind):
    kT,vT = x^T @ w_{k,v}^T on PE (lhsT = x c-tiles -> [s,d] layout, zero transposes)
    expk = exp(kT) (ACT, fp32r); vT staged per head-pair with ones-cols (DVE)
    context accumulated in PSUM: 4 head-pair tiles [128, 130]
      (2 heads per tile, block-diagonal; col 128 accumulates Z = sum_s expk)
    mem_kv folded on host; added via one identity-matmul per pair
    evac: ctx_diag[p] = blockdiag(ctx/Z)^T fp16 SBUF (DVE 32x32 transposes),
      W_eff[p] = ctx_diag^T @ w_out'' (PE), evac'd by GpSimd
  Pass B (8 s-chunks of 512, stage_y pipelined one chunk behind):
    q = w_q @ x (PE, [d,s] layout); expU = exp(q) (ACT)
    Zq[h,s] via block-ones indicator matmul; 1/Zq via reciprocal_approx_fast (DVE)
    expq = expU * (1/Zq) (2 tiles on DVE, 2 on GpSimd to balance engines)
    y = sum_d W_eff_d^T @ expq_d (PE); evac to SBUF fp16 + bias on GpSimd; DMA out
    chunk 0 emitted before the W_eff matmuls so the in-order PE queue is not
    head-of-line blocked on the DVE evac chain
All matmuls fp16 (1 cycle/row, weight loads hidden, ~5e-4 rel err).
"""
import os
import sys

sys.path.insert(0, "/opt/trn_rl_repo")
import numpy as np

import concourse.bass as bass  # noqa: E402
import concourse.bacc as bacc  # noqa: E402
import concourse.mybir as mybir  # noqa: E402
import concourse.tile as tile  # noqa: E402
from concourse import bass_utils  # noqa: E402

B, C, H, W = 16, 256, 64, 64
S = H * W  # 4096
MD, NH, HD, NM = 512, 8, 64, 4
SCALE = HD ** -0.5
EPS = 1e-5
N_CORES = 8
BPC = B // N_CORES
NCT = C // 128
NST = S // 128
NSC = S // 512
NDT = MD // 128
F32 = mybir.dt.float32
F16 = mybir.dt.float16
AF = mybir.ActivationFunctionType

_MODULE_CACHE = {}


def _build_module(has_bk, has_bv):
    nc = bacc.Bacc(
        "TRN2",
        target_bir_lowering=False,
        debug=False,
        enable_asserts=False,
        num_devices=N_CORES,
    )
    # partition-major / chunk-major layouts so DMAs move large contiguous
    # runs per partition (small ct-major rows measured only ~176GB/s)
    x_d = nc.dram_tensor("x", (BPC, NSC, 128, NCT * 512), F16, kind="ExternalInput").ap()
    wqkvT_d = nc.dram_tensor("wqkvT", (128, NCT, 3 * MD), F16, kind="ExternalInput").ap()
    woutT_d = nc.dram_tensor("woutT", (128, NDT, C), F16, kind="ExternalInput").ap()
    bq_d = nc.dram_tensor("bq", (128, NDT), F32, kind="ExternalInput").ap()
    bout_d = nc.dram_tensor("bout", (128, 2), F32, kind="ExternalInput").ap()
    bones_d = nc.dram_tensor("bones", (128, 128), F16, kind="ExternalInput").ap()
    cmem_d = nc.dram_tensor("cmem", (128, NDT, 130), F16, kind="ExternalInput").ap()
    ident_d = nc.dram_tensor("ident", (128, 128), F16, kind="ExternalInput").ap()
    y_d = nc.dram_tensor("y", (BPC, 2, 128, S), F16, kind="ExternalOutput").ap()
    if has_bk or has_bv:
        onesrow_d = nc.dram_tensor("onesrow", (2, 128), F16, kind="ExternalInput").ap()
        bkv_d = nc.dram_tensor("bkv", (2, 2 * MD), F16, kind="ExternalInput").ap()

    with tile.TileContext(nc) as tc, nc.allow_low_precision(reason="fp16 matmul operands"):
        import contextlib

        cstack = contextlib.ExitStack()
        const = cstack.enter_context(tc.tile_pool(name="const", bufs=1))
        xrp = cstack.enter_context(tc.tile_pool(name="xrp", bufs=2))
        work = cstack.enter_context(tc.tile_pool(name="work", bufs=3))
        pool9 = cstack.enter_context(tc.tile_pool(name="pool9", bufs=9))
        pool4 = cstack.enter_context(tc.tile_pool(name="pool4", bufs=4))

        def load_r(shape, src_ap, tag):
            t = const.tile(list(shape), F16, tag=tag, name=tag)
            nc.sync.dma_start(t[:], src_ap)
            return t

        # DMA priority: weights for the first matmuls, then batch-0 x in
        # 512-col chunks (pass A consumes x tile-by-tile, so compute can
        # start as soon as the first chunk lands), then the rest. All on the
        # Sync queue: splitting across DGE queues measured slower.
        wq_r = load_r((128, NCT, 3 * MD), wqkvT_d, "wq")
        x_tiles = []
        for b in range(BPC):
            # chunk-major: [chunk, ct, 512] per partition
            x_rt = xrp.tile([128, NSC, NCT, 512], F16, tag="xr", name=f"xr{b}")
            x_tiles.append(x_rt)
        for c in range(NSC):
            nc.sync.dma_start(x_tiles[0][:, c], x_d[0, c])
        wo_r = load_r((128, NDT, C), woutT_d, "wo")
        bones_r = load_r((128, 128), bones_d, "bones")
        cmem_r = load_r((128, NDT, 130), cmem_d, "cmem")
        ident_r = load_r((128, 128), ident_d, "ident")
        if has_bk or has_bv:
            onesrow_r = load_r((2, 128), onesrow_d, "onesrow")
            bkv_r = load_r((2, 2 * MD), bkv_d, "bkv")

        bq_t = const.tile([128, NDT], F32, tag="bq")
        nc.sync.dma_start(bq_t[:], bq_d)
        bout_t = const.tile([128, 2], F32, tag="bout")
        nc.sync.dma_start(bout_t[:], bout_d)

        for b in range(1, BPC):
            for c in range(NSC):
                nc.sync.dma_start(x_tiles[b][:, c], x_d[b, c])

        # memsets on GpSimd: it is ready ~1.5us before the Vector engine at
        # kernel start, so the warmup matmuls can begin earlier
        zero_r = const.tile([128, 128], F16, tag="zeror")
        nc.gpsimd.memset(zero_r[:], 0.0)

        # persistent staging for v^T per head-pair with ones cols (Z accum)
        vt_bufs = []
        for i in range(2):
            vtb = const.tile([128, NDT, 130], F16, tag=f"vtb{i}", name=f"vtb{i}")
            nc.gpsimd.memset(vtb[:, :, 128:130], 1.0)
            vt_bufs.append(vtb)

        wzero = const.tile([128, 512], F16, tag="wzero")
        nc.gpsimd.memset(wzero[:], 0.0)
        # HAM warmup: dense matmuls with no DMA dependency (overlap input DMAs
        # for the weights + first x chunk, and ramp the PE clock)
        with tc.tile_pool(name="warm", bufs=1, space="PSUM") as pw:
            wps = pw.tile([128, 512], F32, tag="warm")
            for i in range(12):
                nc.tensor.matmul(wps[:], zero_r[:], wzero[:],
                                 start=True, stop=True)

        pp_ctx = tc.tile_pool(name="pp", bufs=1, space="PSUM")
        pp = pp_ctx.__enter__()
        hoisted = {}

        def kv_tile(b, st):
            # one s-tile of the k/v projection + exp/staging for batch b
            x_r = x_tiles[b]
            kps = pp.tile([128, 512], F32, tag=f"s{4 + (2 * st) % 4}",
                          name=f"k_{b}_{st}")
            vps = pp.tile([128, 512], F32, tag=f"s{4 + (2 * st + 1) % 4}",
                          name=f"v_{b}_{st}")
            # interleaved so consecutive matmuls share the stationary x tile
            sc, sub = st // 4, (st % 4) * 128
            for ct in range(NCT):
                last = ct == NCT - 1
                nc.tensor.matmul(
                    kps[:],
                    x_r[:, sc, ct, sub:sub + 128],
                    wq_r[:, ct, MD:2 * MD],
                    start=(ct == 0),
                    stop=(last and not has_bk),
                )
                nc.tensor.matmul(
                    vps[:],
                    x_r[:, sc, ct, sub:sub + 128],
                    wq_r[:, ct, 2 * MD:3 * MD],
                    start=(ct == 0),
                    stop=(last and not has_bv),
                )
            if has_bk:
                nc.tensor.matmul(kps[:], onesrow_r[:], bkv_r[:, 0:MD],
                                 start=False, stop=True)
            if has_bv:
                nc.tensor.matmul(vps[:], onesrow_r[:], bkv_r[:, MD:2 * MD],
                                 start=False, stop=True)
            ek = work.tile([128, 512], F16, tag="ek")
            nc.scalar.activation(ek[:], kps[:], AF.Exp)
            vt = vt_bufs[st % 2]
            nc.vector.tensor_copy(
                vt[:, :, 0:128], vps[:].rearrange("p (g c) -> p g c", g=NDT)
            )
            return (ek, vt, st)

        for b in range(BPC):
            x_r = x_tiles[b]
            # ---- pass A ----
            if True:
                ctxps = [
                    pp.tile([128, 130], F32, tag=f"s{p}", name=f"ctx{p}_{b}")
                    for p in range(NDT)
                ]
                ctxp = [ctxps[p][:] for p in range(NDT)]

                def ctx_mms(ek, vt, st):
                    for p in range(NDT):
                        nc.tensor.matmul(
                            ctxp[p],
                            ek[:, p * 128:(p + 1) * 128],
                            vt[:, p, :],
                            start=(st == 0),
                            stop=False,
                        )

                prev = hoisted.pop(b, None)
                for st in range(1 if prev is not None else 0, NST):
                    cur = kv_tile(b, st)
                    if prev is not None:
                        ctx_mms(*prev)
                    prev = cur
                ctx_mms(*prev)
                for p in range(NDT):
                    nc.tensor.matmul(ctxp[p], ident_r[:], cmem_r[:, p, :],
                                     start=False, stop=True)

                # ---- ctx evac (DVE reads first; ctx banks freed early) ----
                zcat = pool4.tile([128, 4], F32, tag="zcat")
                for p in range(NDT):
                    nc.vector.tensor_copy(zcat[:, p:p + 1], ctxp[p][:, 128:129])
                rz = pool4.tile([128, 4], F32, tag="rz")
                nc.vector.reciprocal_approx_fast(rz[:], zcat[:])
                cds = []
                for p in range(NDT):
                    # cd = (ctx * 1/Z) * blockdiag_mask; bones doubles as mask
                    cd = pool4.tile([128, 128], F16, tag="cd", name=f"cd_{p}_{b}")
                    nc.vector.scalar_tensor_tensor(
                        cd[:], ctxp[p][:, 0:128], rz[:, p:p + 1], bones_r[:],
                        op0=mybir.AluOpType.mult, op1=mybir.AluOpType.mult,
                    )
                    cds.append(cd)

                # ---- pass B ----  y = W_eff^T @ (expU * 1/Zq)
                def q_chunk(s0, w=512, dve_d3=False, post_d1=None):
                    eps = []

                    def zq_norm(d, eu):
                        # Z per head via block-ones indicator matmul, then
                        # normalize; emitted one d late so the PE never waits
                        # on the eu exp
                        zqb = pp.tile([128, w], F32, tag=f"s{4 + d % 2}",
                                      name=f"zqb{b}_{s0}_{d}")
                        nc.tensor.matmul(zqb[:], bones_r[:], eu[:],
                                         start=True, stop=True)
                        rb = pool9.tile([128, w], F32, tag="rb")
                        nc.vector.reciprocal_approx_fast(rb[:], zqb[:])
                        ep = pool9.tile([128, w], F16, tag="ep")
                        # split the normalize multiplies across DVE and GpSimd;
                        # the kernel-final d3 goes on DVE (shorter tail than
                        # GpSimd's ~1.4us op)
                        if d % 2 == 0 or (dve_d3 and d == 3):
                            nc.vector.tensor_mul(ep[:], eu[:], rb[:])
                        else:
                            nc.gpsimd.tensor_mul(ep[:], eu[:], rb[:])
                        eps.append(ep)

                    pend = None
                    for d in range(NDT):
                        # 4-way bank rotation: ctx banks s0/s1 are idle during
                        # pass B, so q never hits a PSUM WAR across chunks
                        qps = pp.tile([128, w], F32, tag=f"s{(6, 7, 0, 1)[d]}",
                                      name=f"q{b}_{s0}_{d}")
                        for ct in range(NCT):
                            nc.tensor.matmul(
                                qps[:],
                                wq_r[:, ct, d * 128:(d + 1) * 128],
                                x_r[:, s0 // 512, ct, 0:w],
                                start=(ct == 0),
                                stop=(ct == NCT - 1),
                            )
                        eu = pool9.tile([128, w], F16, tag="eu")
                        nc.scalar.activation(eu[:], qps[:], AF.Exp,
                                             bias=bq_t[:, d:d + 1])
                        if d == 1 and post_d1 is not None:
                            post_d1()
                        if pend is not None:
                            zq_norm(*pend)
                        pend = (d, eu)
                    zq_norm(*pend)
                    return eps

                def stage_y(s0, w, eps, dve_ot1=False):
                    for ot in range(2):
                        yps = pp.tile([128, w], F32, tag=f"s{2 + ot}",
                                      name=f"y{b}_{s0}_{ot}")
                        for d in range(NDT):
                            nc.tensor.matmul(
                                yps[:],
                                weffs[d][:, ot * 128:(ot + 1) * 128],
                                eps[d][:],
                                start=(d == 0),
                                stop=(d == NDT - 1),
                            )
                        ysb = pool4.tile([128, w], F16, tag="ysb")
                        if ot == 1 and dve_ot1:
                            # kernel-final evac: DVE runs in parallel with
                            # ACT's ot0 evac to shorten the tail
                            nc.vector.tensor_scalar_add(ysb[:], yps[:],
                                                        bout_t[:, ot:ot + 1])
                        else:
                            nc.scalar.activation(ysb[:], yps[:], AF.Identity,
                                                 bias=bout_t[:, ot:ot + 1])
                        nc.sync.dma_start(y_d[b, ot, :, s0:s0 + w], ysb[:])

                # chunk 0 emitted before the W_eff chain keeps PE busy during
                # the DVE evac; the cd transposes slot in after d1's q matmuls
                cdts = []

                def emit_transposes():
                    for p in range(NDT):
                        # cdt = cd^T on the PE (identity-matmul transpose)
                        tps = pp.tile([128, 128], F16, tag=f"s{2 + p % 2}",
                                      name=f"T{b}_{p}")
                        nc.tensor.transpose(tps[:], cds[p][:], ident_r[:])
                        cdt = pool4.tile([128, 128], F16, tag="cdt",
                                         name=f"cdt{b}_{p}")
                        nc.vector.tensor_copy(cdt[:], tps[:])
                        cdts.append(cdt)

                eps0 = q_chunk(0, post_d1=emit_transposes)

                weffs = []
                for p in range(NDT):
                    # W_eff[i, o] = sum_j cd[i, j] * w_out''[j, o]
                    wps = pp.tile([128, C], F32, tag=f"s{2 + p % 2}",
                                  name=f"weff{b}_{p}")
                    nc.tensor.matmul(wps[:], cdts[p][:], wo_r[:, p, :],
                                     start=True, stop=True)
                    weff = pool9.tile([128, C], F16, tag="weff",
                                      name=f"weff_sb{b}_{p}")
                    nc.scalar.copy(weff[:], wps[:])
                    weffs.append(weff)

                st1 = (0, 512, eps0)
                for sc in range(1, NSC):
                    # every batch's final chunk routes d3's multiply to DVE:
                    # the 1.4us GpSimd op otherwise delays the last stage_y
                    # (the kernel tail for the last batch, the transition
                    # bubble for earlier ones)
                    eps = q_chunk(512 * sc, 512, dve_d3=(sc == NSC - 1))
                    if sc == NSC - 1 and b < BPC - 1:
                        # hoist the next batch's first k/v tile so its ek exp
                        # queues ahead of this batch's tail ACT work — removes
                        # the PE stall at the batch transition
                        hoisted[b + 1] = kv_tile(b + 1, 0)
                    stage_y(*st1)
                    st1 = (512 * sc, 512, eps)
                stage_y(*st1, dve_ot1=True)
        pp_ctx.__exit__(None, None, None)
        cstack.close()

    nc.compile()
    return nc


def _prep_consts(w_qkv, b_qkv, mem_kv, w_out, b_out, bn_gamma, bn_beta, bn_mean, bn_var):
    w_qkv = np.asarray(w_qkv, np.float32)
    b_qkv = np.asarray(b_qkv, np.float32)
    mem_kv = np.asarray(mem_kv, np.float32)
    w_out = np.asarray(w_out, np.float32)
    b_out = np.asarray(b_out, np.float32)
    g = np.asarray(bn_gamma, np.float64)
    be = np.asarray(bn_beta, np.float64)
    mu = np.asarray(bn_mean, np.float64)
    var = np.asarray(bn_var, np.float64)

    inv = g / np.sqrt(var + EPS)
    # SCALE (softmax(q) * HD**-0.5) is folded into the output projection
    w_out_f = (w_out.astype(np.float64) * inv[:, None] * SCALE).astype(np.float32)
    b_out_f = ((b_out.astype(np.float64) - mu) * inv + be).astype(np.float32)

    consts = {}
    # partition-major so each tensor loads as ONE DMA with long per-partition
    # contiguous runs (fewer, larger descriptors)
    consts["wqkvT"] = np.ascontiguousarray(
        w_qkv.T.reshape(NCT, 128, 3 * MD).transpose(1, 0, 2)).astype(np.float16)
    consts["woutT"] = np.ascontiguousarray(
        w_out_f.T.reshape(NDT, 128, C).transpose(1, 0, 2)).astype(np.float16)
    consts["bq"] = np.ascontiguousarray(b_qkv[0:MD].reshape(NDT, 128).T)
    consts["bout"] = np.ascontiguousarray(b_out_f.reshape(2, 128).T)

    bones = np.zeros((128, 128), np.float16)
    bones[0:64, 0:64] = 1.0
    bones[64:128, 64:128] = 1.0
    consts["bones"] = bones

    mk = mem_kv[0].astype(np.float64)
    mv = mem_kv[1].astype(np.float64)
    emk = np.exp(mk)
    ctx_mem = np.einsum("him,hjm->hij", emk, mv)
    z_mem = emk.sum(-1)
    # per head-pair layout: [128 rows = d of heads (2p, 2p+1), 130 cols]
    # diag blocks [0:64,0:64] / [64:128,64:128]; col 128 = Z_mem
    cmem = np.zeros((128, NDT, 130), np.float16)
    for p in range(NDT):
        for t in range(2):
            h = 2 * p + t
            r0 = 64 * t
            cmem[r0:r0 + 64, p, 64 * t: 64 * t + 64] = ctx_mem[h]
            cmem[r0:r0 + 64, p, 128] = z_mem[h]
    consts["cmem"] = cmem
    consts["ident"] = np.eye(128, dtype=np.float16)

    has_bk = bool(np.any(b_qkv[MD:2 * MD] != 0))
    has_bv = bool(np.any(b_qkv[2 * MD:] != 0))
    if has_bk or has_bv:
        # K=2 rank-2 form (fp32r wants even dims): ones row + zero row
        onesrow = np.zeros((2, 128), np.float16)
        onesrow[0] = 1.0
        consts["onesrow"] = onesrow
        bkv = np.zeros((2, 2 * MD), np.float16)
        bkv[0] = b_qkv[MD:].astype(np.float16)
        consts["bkv"] = bkv
    return consts, has_bk, has_bv


def kernel(x, w_qkv, b_qkv, mem_kv, w_out, b_out, bn_gamma, bn_beta, bn_mean, bn_var):
    x = np.asarray(x, np.float32)
    consts, has_bk, has_bv = _prep_consts(
        w_qkv, b_qkv, mem_kv, w_out, b_out, bn_gamma, bn_beta, bn_mean, bn_var
    )

    key = (has_bk, has_bv)
    if key not in _MODULE_CACHE:
        _MODULE_CACHE[key] = _build_module(has_bk, has_bv)
    nc = _MODULE_CACHE[key]

    # chunk-major: [b, s-chunk, partition, ct*512] so each 512-col chunk is
    # one DMA with 2KB-per-partition contiguous runs
    x_t = (x.reshape(B, NCT, 128, NSC, 512)
           .transpose(0, 3, 2, 1, 4)
           .reshape(B, NSC, 128, NCT * 512)
           .astype(np.float16))
    in_maps = []
    for c in range(N_CORES):
        m = dict(consts)
        m["x"] = np.ascontiguousarray(x_t[c * BPC:(c + 1) * BPC])
        in_maps.append(m)

    trace = bool(int(os.environ.get("BASS_KERNEL_TRACE", "0")))
    res = bass_utils.run_bass_kernel_spmd(
        nc, in_maps, core_ids=list(range(N_CORES)), trace=trace
    )
    if trace:
        kernel.last_exec_time_ns = res.exec_time_ns
        kernel.last_mean_exec_time_ns = res.mean_exec_time_ns

    y = np.stack([res.results[c]["y"] for c in range(N_CORES)])
    y = y.reshape(B, C, H, W).astype(np.float32)
    return y


# revision 57
# speedup vs baseline: 1.1857x; 1.1857x over previous
"""Trainium2 Bass kernel for LinearSelfAttention3D (16x256x64x64, 8 heads, mem_kv).

Data-parallel over batch: 2 batches per core, 8 cores, identical SPMD program.
Per batch (x viewed [256, 4096] channel-major):
  Pass A (32 s-tiles of 128, ctx matmuls software-pipelined one tile behind):
    kT,vT = x^T @ w_{k,v}^T on PE (lhsT = x c-tiles -> [s,d] layout, zero transposes)
    expk = exp(kT) (ACT, fp32r); vT staged per head-pair with ones-cols (DVE)
    context accumulated in PSUM: 4 head-pair tiles [128, 130]
      (2 heads per tile, block-diagonal; col 128 accumulates Z = sum_s expk)
    mem_kv folded on host; added via one identity-matmul per pair
    evac: ctx_diag[p] = blockdiag(ctx/Z)^T fp16 SBUF (DVE 32x32 transposes),
      W_eff[p] = ctx_diag^T @ w_out'' (PE), evac'd by GpSimd
  Pass B (8 s-chunks of 512, stage_y pipelined one chunk behind):
    q = w_q @ x (PE, [d,s] layout); expU = exp(q) (ACT)
    Zq[h,s] via block-ones indicator matmul; 1/Zq via reciprocal_approx_fast (DVE)
    expq = expU * (1/Zq) (2 tiles on DVE, 2 on GpSimd to balance engines)
    y = sum_d W_eff_d^T @ expq_d (PE); evac to SBUF fp16 + bias on GpSimd; DMA out
    chunk 0 emitted before the W_eff matmuls so the in-order PE queue is not
    head-of-line blocked on the DVE evac chain
All matmuls fp16 (1 cycle/row, weight loads hidden, ~5e-4 rel err).
"""
import os
import sys

sys.path.insert(0, "/opt/trn_rl_repo")
import numpy as np

import concourse.bass as bass  # noqa: E402
import concourse.bacc as bacc  # noqa: E402
import concourse.mybir as mybir  # noqa: E402
import concourse.tile as tile  # noqa: E402
from concourse import bass_utils  # noqa: E402

B, C, H, W = 16, 256, 64, 64
S = H * W  # 4096
MD, NH, HD, NM = 512, 8, 64, 4
SCALE = HD ** -0.5
EPS = 1e-5
N_CORES = 8
BPC = B // N_CORES
NCT = C // 128
NST = S // 128
NSC = S // 512
NDT = MD // 128
F32 = mybir.dt.float32
F16 = mybir.dt.float16
AF = mybir.ActivationFunctionType

_MODULE_CACHE = {}


def _build_module(has_bk, has_bv):
    nc = bacc.Bacc(
        "TRN2",
        target_bir_lowering=False,
        debug=False,
        enable_asserts=False,
        num_devices=N_CORES,
    )
    # partition-major / chunk-major layouts so DMAs move large contiguous
    # runs per partition (small ct-major rows measured only ~176GB/s)
    x_d = nc.dram_tensor("x", (BPC, NSC, 128, NCT * 512), F16, kind="ExternalInput").ap()
    wqkvT_d = nc.dram_tensor("wqkvT", (128, NCT, 3 * MD), F16, kind="ExternalInput").ap()
    woutT_d = nc.dram_tensor("woutT", (128, NDT, C), F16, kind="ExternalInput").ap()
    bq_d = nc.dram_tensor("bq", (128, NDT), F32, kind="ExternalInput").ap()
    bout_d = nc.dram_tensor("bout", (128, 2), F32, kind="ExternalInput").ap()
    bones_d = nc.dram_tensor("bones", (128, 128), F16, kind="ExternalInput").ap()
    cmem_d = nc.dram_tensor("cmem", (128, NDT, 130), F16, kind="ExternalInput").ap()
    ident_d = nc.dram_tensor("ident", (128, 128), F16, kind="ExternalInput").ap()
    y_d = nc.dram_tensor("y", (BPC, 2, 128, S), F16, kind="ExternalOutput").ap()
    if has_bk or has_bv:
        onesrow_d = nc.dram_tensor("onesrow", (2, 128), F16, kind="ExternalInput").ap()
        bkv_d = nc.dram_tensor("bkv", (2, 2 * MD), F16, kind="ExternalInput").ap()

    with tile.TileContext(nc) as tc, nc.allow_low_precision(reason="fp16 matmul operands"):
        import contextlib

        cstack = contextlib.ExitStack()
        const = cstack.enter_context(tc.tile_pool(name="const", bufs=1))
        xrp = cstack.enter_context(tc.tile_pool(name="xrp", bufs=2))
        work = cstack.enter_context(tc.tile_pool(name="work", bufs=3))
        pool9 = cstack.enter_context(tc.tile_pool(name="pool9", bufs=9))
        pool4 = cstack.enter_context(tc.tile_pool(name="pool4", bufs=4))

        def load_r(shape, src_ap, tag):
            t = const.tile(list(shape), F16, tag=tag, name=tag)
            nc.sync.dma_start(t[:], src_ap)
            return t

        # DMA priority: weights for the first matmuls, then batch-0 x in
        # 512-col chunks (pass A consumes x tile-by-tile, so compute can
        # start as soon as the first chunk lands), then the rest. All on the
        # Sync queue: splitting across DGE queues measured slower.
        wq_r = load_r((128, NCT, 3 * MD), wqkvT_d, "wq")
        x_tiles = []
        for b in range(BPC):
            # chunk-major: [chunk, ct, 512] per partition
            x_rt = xrp.tile([128, NSC, NCT, 512], F16, tag="xr", name=f"xr{b}")
            x_tiles.append(x_rt)
        for c in range(NSC):
            nc.sync.dma_start(x_tiles[0][:, c], x_d[0, c])
        wo_r = load_r((128, NDT, C), woutT_d, "wo")
        bones_r = load_r((128, 128), bones_d, "bones")
        cmem_r = load_r((128, NDT, 130), cmem_d, "cmem")
        ident_r = load_r((128, 128), ident_d, "ident")
        if has_bk or has_bv:
            onesrow_r = load_r((2, 128), onesrow_d, "onesrow")
            bkv_r = load_r((2, 2 * MD), bkv_d, "bkv")

        bq_t = const.tile([128, NDT], F32, tag="bq")
        nc.sync.dma_start(bq_t[:], bq_d)
        bout_t = const.tile([128, 2], F32, tag="bout")
        nc.sync.dma_start(bout_t[:], bout_d)

        for b in range(1, BPC):
            for c in range(NSC):
                nc.sync.dma_start(x_tiles[b][:, c], x_d[b, c])

        # memsets on GpSimd: it is ready ~1.5us before the Vector engine at
        # kernel start, so the warmup matmuls can begin earlier
        zero_r = const.tile([128, 128], F16, tag="zeror")
        nc.gpsimd.memset(zero_r[:], 0.0)

        # persistent staging for v^T per head-pair with ones cols (Z accum)
        vt_bufs = []
        for i in range(2):
            vtb = const.tile([128, NDT, 130], F16, tag=f"vtb{i}", name=f"vtb{i}")
            nc.gpsimd.memset(vtb[:, :, 128:130], 1.0)
            vt_bufs.append(vtb)

        wzero = const.tile([128, 512], F16, tag="wzero")
        nc.gpsimd.memset(wzero[:], 0.0)
        # HAM warmup: dense matmuls with no DMA dependency (overlap input DMAs
        # for the weights + first x chunk, and ramp the PE clock)
        with tc.tile_pool(name="warm", bufs=1, space="PSUM") as pw:
            wps = pw.tile([128, 512], F32, tag="warm")
            for i in range(12):
                nc.tensor.matmul(wps[:], zero_r[:], wzero[:],
                                 start=True, stop=True)

        pp_ctx = tc.tile_pool(name="pp", bufs=1, space="PSUM")
        pp = pp_ctx.__enter__()
        hoisted = {}

        def kv_tile(b, st):
            # one s-tile of the k/v projection + exp/staging for batch b
            x_r = x_tiles[b]
            kps = pp.tile([128, 512], F32, tag=f"s{4 + (2 * st) % 4}",
                          name=f"k_{b}_{st}")
            vps = pp.tile([128, 512], F32, tag=f"s{4 + (2 * st + 1) % 4}",
                          name=f"v_{b}_{st}")
            # interleaved so consecutive matmuls share the stationary x tile
            sc, sub = st // 4, (st % 4) * 128
            for ct in range(NCT):
                last = ct == NCT - 1
                nc.tensor.matmul(
                    kps[:],
                    x_r[:, sc, ct, sub:sub + 128],
                    wq_r[:, ct, MD:2 * MD],
                    start=(ct == 0),
                    stop=(last and not has_bk),
                )
                nc.tensor.matmul(
                    vps[:],
                    x_r[:, sc, ct, sub:sub + 128],
                    wq_r[:, ct, 2 * MD:3 * MD],
                    start=(ct == 0),
                    stop=(last and not has_bv),
                )
            if has_bk:
                nc.tensor.matmul(kps[:], onesrow_r[:], bkv_r[:, 0:MD],
                                 start=False, stop=True)
            if has_bv:
                nc.tensor.matmul(vps[:], onesrow_r[:], bkv_r[:, MD:2 * MD],
                                 start=False, stop=True)
            ek = work.tile([128, 512], F16, tag="ek")
            nc.scalar.activation(ek[:], kps[:], AF.Exp)
            vt = vt_bufs[st % 2]
            nc.vector.tensor_copy(
                vt[:, :, 0:128], vps[:].rearrange("p (g c) -> p g c", g=NDT)
            )
            return (ek, vt, st)

        for b in range(BPC):
            x_r = x_tiles[b]
            # ---- pass A ----
            if True:
                ctxps = [
                    pp.tile([128, 130], F32, tag=f"s{p}", name=f"ctx{p}_{b}")
                    for p in range(NDT)
                ]
                ctxp = [ctxps[p][:] for p in range(NDT)]

                def ctx_mms(ek, vt, st):
                    for p in range(NDT):
                        nc.tensor.matmul(
                            ctxp[p],
                            ek[:, p * 128:(p + 1) * 128],
                            vt[:, p, :],
                            start=(st == 0),
                            stop=False,
                        )

                prev = hoisted.pop(b, None)
                for st in range(1 if prev is not None else 0, NST):
                    cur = kv_tile(b, st)
                    if prev is not None:
                        ctx_mms(*prev)
                    prev = cur
                ctx_mms(*prev)
                for p in range(NDT):
                    nc.tensor.matmul(ctxp[p], ident_r[:], cmem_r[:, p, :],
                                     start=False, stop=True)

                # ---- ctx evac (DVE reads first; ctx banks freed early) ----
                zcat = pool4.tile([128, 4], F32, tag="zcat")
                for p in range(NDT):
                    nc.vector.tensor_copy(zcat[:, p:p + 1], ctxp[p][:, 128:129])
                rz = pool4.tile([128, 4], F32, tag="rz")
                nc.vector.reciprocal_approx_fast(rz[:], zcat[:])
                cds = []
                for p in range(NDT):
                    # cd = (ctx * 1/Z) * blockdiag_mask; bones doubles as mask
                    cd = pool4.tile([128, 128], F16, tag="cd", name=f"cd_{p}_{b}")
                    nc.vector.scalar_tensor_tensor(
                        cd[:], ctxp[p][:, 0:128], rz[:, p:p + 1], bones_r[:],
                        op0=mybir.AluOpType.mult, op1=mybir.AluOpType.mult,
                    )
                    cds.append(cd)

                # ---- pass B ----  y = W_eff^T @ (expU * 1/Zq)
                def q_chunk(s0, w=512, dve_d3=False, post_d1=None):
                    eps = []

                    def zq_norm(d, eu):
                        # Z per head via block-ones indicator matmul, then
                        # normalize; emitted one d late so the PE never waits
                        # on the eu exp
                        zqb = pp.tile([128, w], F32, tag=f"s{4 + d % 2}",
                                      name=f"zqb{b}_{s0}_{d}")
                        nc.tensor.matmul(zqb[:], bones_r[:], eu[:],
                                         start=True, stop=True)
                        rb = pool9.tile([128, w], F32, tag="rb")
                        nc.vector.reciprocal_approx_fast(rb[:], zqb[:])
                        ep = pool9.tile([128, w], F16, tag="ep")
                        # split the normalize multiplies across DVE and GpSimd;
                        # the kernel-final d3 goes on DVE (shorter tail than
                        # GpSimd's ~1.4us op)
                        if d % 2 == 0 or (dve_d3 and d == 3):
                            nc.vector.tensor_mul(ep[:], eu[:], rb[:])
                        else:
                            nc.gpsimd.tensor_mul(ep[:], eu[:], rb[:])
                        eps.append(ep)

                    pend = None
                    for d in range(NDT):
                        # 4-way bank rotation: ctx banks s0/s1 are idle during
                        # pass B, so q never hits a PSUM WAR across chunks
                        qps = pp.tile([128, w], F32, tag=f"s{(6, 7, 0, 1)[d]}",
                                      name=f"q{b}_{s0}_{d}")
                        for ct in range(NCT):
                            nc.tensor.matmul(
                                qps[:],
                                wq_r[:, ct, d * 128:(d + 1) * 128],
                                x_r[:, s0 // 512, ct, 0:w],
                                start=(ct == 0),
                                stop=(ct == NCT - 1),
                            )
                        eu = pool9.tile([128, w], F16, tag="eu")
                        nc.scalar.activation(eu[:], qps[:], AF.Exp,
                                             bias=bq_t[:, d:d + 1])
                        if d == 1 and post_d1 is not None:
                            post_d1()
                        if pend is not None:
                            zq_norm(*pend)
                        pend = (d, eu)
                    zq_norm(*pend)
                    return eps

                def stage_y(s0, w, eps, dve_ot1=False):
                    for ot in range(2):
                        yps = pp.tile([128, w], F32, tag=f"s{2 + ot}",
                                      name=f"y{b}_{s0}_{ot}")
                        for d in range(NDT):
                            nc.tensor.matmul(
                                yps[:],
                                weffs[d][:, ot * 128:(ot + 1) * 128],
                                eps[d][:],
                                start=(d == 0),
                                stop=(d == NDT - 1),
                            )
                        ysb = pool4.tile([128, w], F16, tag="ysb")
                        if ot == 1 and dve_ot1:
                            # kernel-final evac: DVE runs in parallel with
                            # ACT's ot0 evac to shorten the tail
                            nc.vector.tensor_scalar_add(ysb[:], yps[:],
                                                        bout_t[:, ot:ot + 1])
                        else:
                            nc.scalar.activation(ysb[:], yps[:], AF.Identity,
                                                 bias=bout_t[:, ot:ot + 1])
                        nc.sync.dma_start(y_d[b, ot, :, s0:s0 + w], ysb[:])

                # chunk 0 emitted before the W_eff chain keeps PE busy during
                # the DVE evac; the cd transposes slot in after d1's q matmuls
                cdts = []

                def emit_transposes():
                    for p in range(NDT):
                        # cdt = cd^T on the PE (identity-matmul transpose)
                        tps = pp.tile([128, 128], F16, tag=f"s{2 + p % 2}",
                                      name=f"T{b}_{p}")
                        nc.tensor.transpose(tps[:], cds[p][:], ident_r[:])
                        cdt = pool4.tile([128, 128], F16, tag="cdt",
                                         name=f"cdt{b}_{p}")
                        nc.vector.tensor_copy(cdt[:], tps[:])
                        cdts.append(cdt)

                eps0 = q_chunk(0, post_d1=emit_transposes)

                weffs = []
                for p in range(NDT):
                    # W_eff[i, o] = sum_j cd[i, j] * w_out''[j, o]
                    wps = pp.tile([128, C], F32, tag=f"s{2 + p % 2}",
                                  name=f"weff{b}_{p}")
                    nc.tensor.matmul(wps[:], cdts[p][:], wo_r[:, p, :],
                                     start=True, stop=True)
                    weff = pool9.tile([128, C], F16, tag="weff",
                                      name=f"weff_sb{b}_{p}")
                    nc.scalar.copy(weff[:], wps[:])
                    weffs.append(weff)

                last = b == BPC - 1
                st1 = (0, 512, eps0)
                for sc in range(1, NSC):
                    eps = q_chunk(512 * sc, 512,
                                  dve_d3=(last and sc == NSC - 1))
                    if sc == NSC - 1 and not last:
                        # hoist the next batch's first k/v tile so its ek exp
                        # queues ahead of this batch's tail ACT work — removes
                        # the PE stall at the batch transition
                        hoisted[b + 1] = kv_tile(b + 1, 0)
                    stage_y(*st1)
                    st1 = (512 * sc, 512, eps)
                stage_y(*st1, dve_ot1=last)
        pp_ctx.__exit__(None, None, None)
        cstack.close()

    nc.compile()
    return nc


def _prep_consts(w_qkv, b_qkv, mem_kv, w_out, b_out, bn_gamma, bn_beta, bn_mean, bn_var):
    w_qkv = np.asarray(w_qkv, np.float32)
    b_qkv = np.asarray(b_qkv, np.float32)
    mem_kv = np.asarray(mem_kv, np.float32)
    w_out = np.asarray(w_out, np.float32)
    b_out = np.asarray(b_out, np.float32)
    g = np.asarray(bn_gamma, np.float64)
    be = np.asarray(bn_beta, np.float64)
    mu = np.asarray(bn_mean, np.float64)
    var = np.asarray(bn_var, np.float64)

    inv = g / np.sqrt(var + EPS)
    # SCALE (softmax(q) * HD**-0.5) is folded into the output projection
    w_out_f = (w_out.astype(np.float64) * inv[:, None] * SCALE).astype(np.float32)
    b_out_f = ((b_out.astype(np.float64) - mu) * inv + be).astype(np.float32)

    consts = {}
    # partition-major so each tensor loads as ONE DMA with long per-partition
    # contiguous runs (fewer, larger descriptors)
    consts["wqkvT"] = np.ascontiguousarray(
        w_qkv.T.reshape(NCT, 128, 3 * MD).transpose(1, 0, 2)).astype(np.float16)
    consts["woutT"] = np.ascontiguousarray(
        w_out_f.T.reshape(NDT, 128, C).transpose(1, 0, 2)).astype(np.float16)
    consts["bq"] = np.ascontiguousarray(b_qkv[0:MD].reshape(NDT, 128).T)
    consts["bout"] = np.ascontiguousarray(b_out_f.reshape(2, 128).T)

    bones = np.zeros((128, 128), np.float16)
    bones[0:64, 0:64] = 1.0
    bones[64:128, 64:128] = 1.0
    consts["bones"] = bones

    mk = mem_kv[0].astype(np.float64)
    mv = mem_kv[1].astype(np.float64)
    emk = np.exp(mk)
    ctx_mem = np.einsum("him,hjm->hij", emk, mv)
    z_mem = emk.sum(-1)
    # per head-pair layout: [128 rows = d of heads (2p, 2p+1), 130 cols]
    # diag blocks [0:64,0:64] / [64:128,64:128]; col 128 = Z_mem
    cmem = np.zeros((128, NDT, 130), np.float16)
    for p in range(NDT):
        for t in range(2):
            h = 2 * p + t
            r0 = 64 * t
            cmem[r0:r0 + 64, p, 64 * t: 64 * t + 64] = ctx_mem[h]
            cmem[r0:r0 + 64, p, 128] = z_mem[h]
    consts["cmem"] = cmem
    consts["ident"] = np.eye(128, dtype=np.float16)

    has_bk = bool(np.any(b_qkv[MD:2 * MD] != 0))
    has_bv = bool(np.any(b_qkv[2 * MD:] != 0))
    if has_bk or has_bv:
        # K=2 rank-2 form (fp32r wants even dims): ones row + zero row
        onesrow = np.zeros((2, 128), np.float16)
        onesrow[0] = 1.0
        consts["onesrow"] = onesrow
        bkv = np.zeros((2, 2 * MD), np.float16)
        bkv[0] = b_qkv[MD:].astype(np.float16)
        consts["bkv"] = bkv
    return consts, has_bk, has_bv


def kernel(x, w_qkv, b_qkv, mem_kv, w_out, b_out, bn_gamma, bn_beta, bn_mean, bn_var):
    x = np.asarray(x, np.float32)
    consts, has_bk, has_bv = _prep_consts(
        w_qkv, b_qkv, mem_kv, w_out, b_out, bn_gamma, bn_beta, bn_mean, bn_var
    )

    key = (has_bk, has_bv)
    if key not in _MODULE_CACHE:
        _MODULE_CACHE[key] = _build_module(has_bk, has_bv)
    nc = _MODULE_CACHE[key]

    # chunk-major: [b, s-chunk, partition, ct*512] so each 512-col chunk is
    # one DMA with 2KB-per-partition contiguous runs
    x_t = (x.reshape(B, NCT, 128, NSC, 512)
           .transpose(0, 3, 2, 1, 4)
           .reshape(B, NSC, 128, NCT * 512)
           .astype(np.float16))
    in_maps = []
    for c in range(N_CORES):
        m = dict(consts)
        m["x"] = np.ascontiguousarray(x_t[c * BPC:(c + 1) * BPC])
        in_maps.append(m)

    trace = bool(int(os.environ.get("BASS_KERNEL_TRACE", "0")))
    res = bass_utils.run_bass_kernel_spmd(
        nc, in_maps, core_ids=list(range(N_CORES)), trace=trace
    )
    if trace:
        kernel.last_exec_time_ns = res.exec_time_ns
        kernel.last_mean_exec_time_ns = res.mean_exec_time_ns

    y = np.stack([res.results[c]["y"] for c in range(N_CORES)])
    y = y.reshape(B, C, H, W).astype(np.float32)
    return y
